# revision 32
# baseline (speedup 1.0000x reference)
"""Trainium2 Bass kernel for AdjacencyMatchingLoss.

Math: adj_score[b,e] = P[b,i_e,:] @ A @ P[b,j_e,:]  with A = (d_hw==1).
Let W[i,j] = sum_e w_e * 1[i_e=i] * 1[j_e=j]   (weighted pair histogram)
Then  total_adj = sum_ij W[i,j] * mean_b (P_b A P_b^T)[i,j]
               = (1/B) * sum_b < P_b^T W P_b , A >
Per core: shard edges (E/8), build W via one-hot matmuls on the
TensorEngine (one-hot construction split across DVE/gpsimd/ACT), compute
C = sum_b P_b^T W P_b (layouts work out so no transposes are ever
needed), reduce <C, -A/8> and the local weight sum to [128,2] partials;
host sums partials over partitions and cores and divides.

W is accumulated in two halves so the first half's U = W^T P_b and
C += P_b^T W P_b matmuls overlap the second half's one-hot build.
"""

import os
import sys

import numpy as np

for _p in ("/opt/trn_rl_repo",):
    if os.path.isdir(_p) and _p not in sys.path:
        sys.path.insert(0, _p)

B, NL, NQ, E = 8, 128, 128, 50000
NCORES = 8
ESH = E // NCORES            # 6250 edges per core
CHUNKS = (ESH + 127) // 128  # 49
EPAD = CHUNKS * 128          # 6272
SPLIT_W = False
META_W = 768                 # i16 words/partition: 392 pairs | 98 w | 256 d | pad

_BUILT = None


def _emit_body(nc, sp, pp, tensors):
    import concourse.mybir as mybir

    f32 = mybir.dt.float32
    bf16 = mybir.dt.bfloat16
    i32 = mybir.dt.int32
    i16 = mybir.dt.int16
    EQ = mybir.AluOpType.is_equal
    MUL = mybir.AluOpType.mult
    ADD = mybir.AluOpType.add
    ABS = mybir.ActivationFunctionType.Abs
    RELU = mybir.ActivationFunctionType.Relu
    P_d, meta_d, o_d = tensors

    Pf = sp.tile([128, B * NQ], f32)
    Pb = sp.tile([128, B * NQ], bf16)
    meta = sp.tile([128, META_W], i16)
    Asc = sp.tile([128, NQ], f32)
    idx = sp.tile([128, 2 * CHUNKS], f32)   # interleaved [c][i,j]
    wNeg = sp.tile([128, CHUNKS], f32)
    iot = sp.tile([128, 128], bf16)
    OhJ = sp.tile([128, EPAD], bf16)
    OhIW = sp.tile([128, EPAD], bf16)
    WsbA = sp.tile([128, 128], bf16)
    WsbB = sp.tile([128, 128], bf16)
    UsbA = sp.tile([128, B * NQ], bf16)
    UsbB = sp.tile([128, B * NQ], bf16)
    prt = sp.tile([128, 2], f32)
    scr = sp.tile([128, NQ], f32)

    WpsA = pp.tile([128, 128], f32)
    WpsB = pp.tile([128, 128], f32)
    Up0 = pp.tile([128, 512], f32)
    Up1 = pp.tile([128, 512], f32)
    Cps = pp.tile([128, 128], f32)

    # ---- loads ----
    # pairs+w words first (they gate the one-hot phase); the d_hw words
    # ride in the same packed tensor but are only needed at the tail.
    nc.sync.dma_start(out=meta[:, 0:490], in_=meta_d.ap()[:, 0:490])
    P_src = P_d.ap().rearrange("b l q -> l b q")
    Pf3 = Pf[:].rearrange("l (b q) -> l b q", q=NQ)
    nc.sync.dma_start(out=Pf3[:, 0:4, :], in_=P_src[:, 0:4, :])
    nc.sync.dma_start(out=Pf3[:, 4:8, :], in_=P_src[:, 4:8, :])
    nc.sync.dma_start(out=meta[:, 490:746], in_=meta_d.ap()[:, 490:746])

    # views into the packed meta row
    prs3 = meta[:, 0:392].rearrange("p (c k) -> p c k", k=8)
    wT = meta[:, 392:490].bitcast(f32)          # [128, 49] f32
    dsb = meta[:, 490:746].bitcast(i32)         # [128, 128] i32

    # ---- prep ----
    nc.gpsimd.iota(
        iot[:],
        pattern=[[1, 128]],
        base=0,
        channel_multiplier=0,
        allow_small_or_imprecise_dtypes=True,
    )
    # both index columns (i at i16 word 0, j at word 4) in one strided copy
    nc.vector.tensor_copy(
        out=idx[:].rearrange("p (c k) -> p c k", k=2),
        in_=meta[:, 0:392].rearrange("p (c k u) -> p c k u", k=2, u=4)[:, :, :, 0:1],
    )
    nc.gpsimd.tensor_scalar(
        out=wNeg[:], in0=wT, scalar1=-1.0, scalar2=None, op0=MUL
    )
    nc.vector.tensor_reduce(
        out=prt[:, 1:2], in_=wT, axis=mybir.AxisListType.X, op=ADD
    )

    def idxI(c):
        return idx[:, 2 * c : 2 * c + 1]

    def idxJ(c):
        return idx[:, 2 * c + 1 : 2 * c + 2]

    # ---- one-hots + W accumulation (two halves) + overlapped U/C ----
    if SPLIT_W:
        ACT_CHUNKS = {5, 15, 25, 35}
        POOL_CHUNKS = {3, 8, 13, 18, 23, 28, 33, 38, 43}
        HALF = 25
    else:
        ACT_CHUNKS = {5, 15, 25, 35}
        POOL_CHUNKS = {3, 8, 13, 18, 23, 28, 33, 38, 43}
        HALF = CHUNKS

    def emit_chunk(c):
        sl = slice(c * 128, (c + 1) * 128)
        if c in ACT_CHUNKS:
            tmpJ = sp.tile([128, 128], bf16, name=f"tmpJ{c}")
            tmpI = sp.tile([128, 128], bf16, name=f"tmpI{c}")
            nc.scalar.activation(
                out=tmpJ[:], in_=iot[:], func=ABS, bias=idxJ(c), scale=-1.0
            )
            nc.scalar.activation(
                out=OhJ[:, sl], in_=tmpJ[:], func=RELU, bias=1.0, scale=-1.0
            )
            nc.scalar.activation(
                out=tmpI[:], in_=iot[:], func=ABS, bias=idxI(c), scale=-1.0
            )
            nc.scalar.activation(
                out=OhIW[:, sl], in_=tmpI[:], func=RELU,
                bias=wT[:, c : c + 1], scale=wNeg[:, c : c + 1],
            )
        else:
            eng = nc.gpsimd if c in POOL_CHUNKS else nc.vector
            eng.tensor_scalar(
                out=OhJ[:, sl], in0=iot[:], scalar1=idxJ(c), scalar2=None, op0=EQ
            )
            eng.tensor_scalar(
                out=OhIW[:, sl], in0=iot[:], scalar1=idxI(c),
                scalar2=wT[:, c : c + 1], op0=EQ, op1=MUL,
            )
        Wp = WpsA if c < HALF else WpsB
        nc.tensor.matmul(
            Wp[:],
            lhsT=OhIW[:, sl],
            rhs=OhJ[:, sl],
            start=(c in (0, HALF)),
            stop=(c in (HALF - 1, CHUNKS - 1)),
        )

    def emit_uc(Wsb, Wps, Usb, first, last):
        nc.vector.tensor_copy(out=Wsb[:], in_=Wps[:])
        nc.tensor.matmul(
            Up0[:], lhsT=Wsb[:], rhs=Pb[:, 0:512], start=True, stop=True
        )
        nc.tensor.matmul(
            Up1[:], lhsT=Wsb[:], rhs=Pb[:, 512:1024], start=True, stop=True
        )
        if not last:
            # phase still running: keep DVE free, ACT absorbs both copies
            nc.scalar.copy(out=Usb[:, 0:512], in_=Up0[:])
            nc.scalar.copy(out=Usb[:, 512:1024], in_=Up1[:])
        else:
            nc.vector.tensor_copy(out=Usb[:, 0:512], in_=Up0[:])
            nc.scalar.copy(out=Usb[:, 512:1024], in_=Up1[:])
        for b in range(B):
            sl = slice(b * 128, (b + 1) * 128)
            nc.tensor.matmul(
                Cps[:],
                lhsT=Usb[:, sl],
                rhs=Pb[:, sl],
                start=(first and b == 0),
                stop=(last and b == B - 1),
            )

    pb_at = (18, 20) if SPLIT_W else (40, 43)
    for c in range(CHUNKS):
        if c == pb_at[0]:
            # P f32->bf16 slipped into the one-hot stream, in time for the
            # first U matmuls
            nc.vector.tensor_copy(out=Pb[:, 0:512], in_=Pf[:, 0:512])
        if c == pb_at[1]:
            nc.gpsimd.tensor_copy(out=Pb[:, 512:1024], in_=Pf[:, 512:1024])
        emit_chunk(c)
        if SPLIT_W and c == HALF - 1:
            emit_uc(WsbA, WpsA, UsbA, first=True, last=False)
    if SPLIT_W:
        emit_uc(WsbB, WpsB, UsbB, first=False, last=True)
    else:
        emit_uc(WsbA, WpsA, UsbA, first=True, last=True)

    # A_scaled = -(1/8) * (d_hw == 1); folds sign + batch-mean
    nc.gpsimd.tensor_scalar(
        out=Asc[:], in0=dsb, scalar1=1, scalar2=-0.125, op0=EQ, op1=MUL
    )

    # ---- partials: [ <C, -A/8> , sum(w) ] ----
    nc.vector.tensor_tensor(out=scr[:], in0=Cps[:], in1=Asc[:], op=MUL)
    nc.vector.tensor_reduce(
        out=prt[:, 0:1], in_=scr[:], axis=mybir.AxisListType.X, op=ADD
    )
    # partition + cross-core reduction of the [128,2] partials on host
    nc.sync.dma_start(out=o_d.ap(), in_=prt[:])


def _build(reps=1):
    import concourse.bacc as bacc
    import concourse.mybir as mybir
    import concourse.tile as tile

    f32 = mybir.dt.float32
    i16 = mybir.dt.int16

    nc = bacc.Bacc("TRN2", target_bir_lowering=False, debug=False, num_devices=NCORES)

    P_d = nc.dram_tensor("p_in", [B, NL, NQ], f32, kind="ExternalInput")
    meta_d = nc.dram_tensor("meta_in", [128, META_W], i16, kind="ExternalInput")
    o_d = nc.dram_tensor("out", [128, 2], f32, kind="ExternalOutput")

    with tile.TileContext(nc) as tc:
        with (
            tc.tile_pool(name="sbuf", bufs=1) as sp,
            tc.tile_pool(name="psum", bufs=1, space="PSUM") as pp,
        ):
            for _ in range(reps):
                _emit_body(nc, sp, pp, (P_d, meta_d, o_d))

    nc.compile()
    return nc


def _get_built():
    global _BUILT
    if _BUILT is None:
        _BUILT = _build()
    return _BUILT


def _shard_inputs(P, d_hw, circuit_edge_pairs, circuit_edge_weights):
    P = np.ascontiguousarray(np.asarray(P, dtype=np.float32))
    d_hw = np.ascontiguousarray(np.asarray(d_hw, dtype=np.int32))
    pairs = np.asarray(circuit_edge_pairs).astype(np.int64, copy=False)
    w = np.asarray(circuit_edge_weights, dtype=np.float32)

    pairs_pad = np.zeros((NCORES, EPAD, 2), dtype=np.int64)
    w_pad = np.zeros((NCORES, EPAD), dtype=np.float32)
    pairs_pad[:, :ESH] = pairs.reshape(NCORES, ESH, 2)
    w_pad[:, :ESH] = w.reshape(NCORES, ESH)

    # packed per-partition row: 392 i16 of pairs | 98 i16 (49 f32 w) |
    # 256 i16 (128 i32 d row) | pad to 768
    meta = np.zeros((NCORES, 128, META_W), dtype=np.int16)
    meta[:, :, 0:392] = pairs_pad.view(np.int16).reshape(NCORES, 128, 392)
    meta[:, :, 392:490] = w_pad.view(np.int16).reshape(NCORES, 128, 98)
    meta[:, :, 490:746] = d_hw.view(np.int16).reshape(128, 256)[None]

    return [
        {"p_in": P, "meta_in": np.ascontiguousarray(meta[i])}
        for i in range(NCORES)
    ]


def _combine(results):
    parts = np.stack([np.asarray(results[i]["out"]) for i in range(NCORES)])
    numer = float(parts[:, :, 0].astype(np.float64).sum())
    wsum = float(parts[:, :, 1].astype(np.float64).sum())
    return np.float32(numer / max(wsum, 1e-8))


def kernel(P, d_hw, circuit_edge_pairs, circuit_edge_weights, _want_results=False):
    from concourse.bass_utils import run_bass_kernel_spmd

    nc = _get_built()
    in_maps = _shard_inputs(P, d_hw, circuit_edge_pairs, circuit_edge_weights)
    res = run_bass_kernel_spmd(nc, in_maps, core_ids=list(range(NCORES)))
    out = _combine(res.results)
    if _want_results:
        return out, res
    return out


# revision 33
# speedup vs baseline: 1.2260x; 1.2260x over previous
"""Trainium2 Bass kernel for AdjacencyMatchingLoss.

Math: adj_score[b,e] = P[b,i_e,:] @ A @ P[b,j_e,:]  with A = (d_hw==1).
Let W[i,j] = sum_e w_e * 1[i_e=i] * 1[j_e=j]   (weighted pair histogram)
Then  total_adj = sum_ij W[i,j] * mean_b (P_b A P_b^T)[i,j]
               = (1/B) * sum_b < P_b^T W P_b , A >
Per core: shard edges (E/8), build W via one-hot matmuls on the
TensorEngine (one-hot construction split across DVE/gpsimd/ACT), compute
C = sum_b P_b^T W P_b (layouts work out so no transposes are ever
needed), reduce <C, -A/8> and the local weight sum to [128,2] partials;
host sums partials over partitions and cores and divides.

W is accumulated in two halves so the first half's U = W^T P_b and
C += P_b^T W P_b matmuls overlap the second half's one-hot build.
"""

import os
import sys

import numpy as np

for _p in ("/opt/trn_rl_repo",):
    if os.path.isdir(_p) and _p not in sys.path:
        sys.path.insert(0, _p)

B, NL, NQ, E = 8, 128, 128, 50000
NCORES = 8
ESH = E // NCORES            # 6250 edges per core
CHUNKS = (ESH + 127) // 128  # 49
EPAD = CHUNKS * 128          # 6272
SPLIT_W = False
META_W = 768                 # i16 words/partition: 392 pairs | 98 w | 256 d | pad

_BUILT = None


def _emit_body(nc, sp, pp, tensors):
    import concourse.mybir as mybir

    f32 = mybir.dt.float32
    bf16 = mybir.dt.bfloat16
    i32 = mybir.dt.int32
    i16 = mybir.dt.int16
    EQ = mybir.AluOpType.is_equal
    MUL = mybir.AluOpType.mult
    ADD = mybir.AluOpType.add
    ABS = mybir.ActivationFunctionType.Abs
    RELU = mybir.ActivationFunctionType.Relu
    P_d, meta_d, o_d = tensors

    Pf = sp.tile([128, B * NQ], f32)
    Pb = sp.tile([128, B * NQ], bf16)
    meta = sp.tile([128, META_W], i16)
    Asc = sp.tile([128, NQ], f32)
    idx = sp.tile([128, 2 * CHUNKS], f32)   # interleaved [c][i,j]
    wNeg = sp.tile([128, CHUNKS], f32)
    iot = sp.tile([128, 128], bf16)
    OhJ = sp.tile([128, EPAD], bf16)
    OhIW = sp.tile([128, EPAD], bf16)
    WsbA = sp.tile([128, 128], bf16)
    UsbA = sp.tile([128, B * NQ], bf16)
    if SPLIT_W:
        WsbB = sp.tile([128, 128], bf16)
        UsbB = sp.tile([128, B * NQ], bf16)
    prt = sp.tile([128, 2], f32)
    scr = sp.tile([128, NQ], f32)

    WpsA = pp.tile([128, 128], f32)
    if SPLIT_W:
        WpsB = pp.tile([128, 128], f32)
    else:
        WpsB = WpsA
    Up0 = pp.tile([128, 512], f32)
    Up1 = pp.tile([128, 512], f32)
    Cps = pp.tile([128, 128], f32)

    # ---- loads ----
    # pairs+w words first (they gate the one-hot phase); the d_hw words
    # ride in the same packed tensor but are only needed at the tail.
    nc.sync.dma_start(out=meta[:, 0:490], in_=meta_d.ap()[:, 0:490])
    P_src = P_d.ap().rearrange("b l q -> l b q")
    Pf3 = Pf[:].rearrange("l (b q) -> l b q", q=NQ)
    nc.sync.dma_start(out=Pf3[:, 0:4, :], in_=P_src[:, 0:4, :])
    nc.sync.dma_start(out=Pf3[:, 4:8, :], in_=P_src[:, 4:8, :])
    nc.sync.dma_start(out=meta[:, 490:746], in_=meta_d.ap()[:, 490:746])

    # views into the packed meta row
    prs3 = meta[:, 0:392].rearrange("p (c k) -> p c k", k=8)
    wT = meta[:, 392:490].bitcast(f32)          # [128, 49] f32
    dsb = meta[:, 490:746].bitcast(i32)         # [128, 128] i32

    # ---- prep ----
    nc.gpsimd.iota(
        iot[:],
        pattern=[[1, 128]],
        base=0,
        channel_multiplier=0,
        allow_small_or_imprecise_dtypes=True,
    )
    # both index columns (i at i16 word 0, j at word 4) in one strided copy
    nc.vector.tensor_copy(
        out=idx[:].rearrange("p (c k) -> p c k", k=2),
        in_=meta[:, 0:392].rearrange("p (c k u) -> p c k u", k=2, u=4)[:, :, :, 0:1],
    )
    nc.gpsimd.tensor_scalar(
        out=wNeg[:], in0=wT, scalar1=-1.0, scalar2=None, op0=MUL
    )
    nc.vector.tensor_reduce(
        out=prt[:, 1:2], in_=wT, axis=mybir.AxisListType.X, op=ADD
    )

    def idxI(c):
        return idx[:, 2 * c : 2 * c + 1]

    def idxJ(c):
        return idx[:, 2 * c + 1 : 2 * c + 2]

    # ---- one-hots + W accumulation (two halves) + overlapped U/C ----
    if SPLIT_W:
        ACT_CHUNKS = {5, 15, 25, 35}
        POOL_CHUNKS = {3, 8, 13, 18, 23, 28, 33, 38, 43}
        HALF = 25
    else:
        ACT_CHUNKS = {5, 15, 25, 35}
        POOL_CHUNKS = {3, 8, 13, 18, 23, 28, 33, 38, 43}
        HALF = CHUNKS

    def emit_chunk(c):
        sl = slice(c * 128, (c + 1) * 128)
        if c in ACT_CHUNKS:
            tmpJ = sp.tile([128, 128], bf16, name=f"tmpJ{c}")
            tmpI = sp.tile([128, 128], bf16, name=f"tmpI{c}")
            nc.scalar.activation(
                out=tmpJ[:], in_=iot[:], func=ABS, bias=idxJ(c), scale=-1.0
            )
            nc.scalar.activation(
                out=OhJ[:, sl], in_=tmpJ[:], func=RELU, bias=1.0, scale=-1.0
            )
            nc.scalar.activation(
                out=tmpI[:], in_=iot[:], func=ABS, bias=idxI(c), scale=-1.0
            )
            nc.scalar.activation(
                out=OhIW[:, sl], in_=tmpI[:], func=RELU,
                bias=wT[:, c : c + 1], scale=wNeg[:, c : c + 1],
            )
        else:
            eng = nc.gpsimd if c in POOL_CHUNKS else nc.vector
            eng.tensor_scalar(
                out=OhJ[:, sl], in0=iot[:], scalar1=idxJ(c), scalar2=None, op0=EQ
            )
            eng.tensor_scalar(
                out=OhIW[:, sl], in0=iot[:], scalar1=idxI(c),
                scalar2=wT[:, c : c + 1], op0=EQ, op1=MUL,
            )
        Wp = WpsA if c < HALF else WpsB
        nc.tensor.matmul(
            Wp[:],
            lhsT=OhIW[:, sl],
            rhs=OhJ[:, sl],
            start=(c in (0, HALF)),
            stop=(c in (HALF - 1, CHUNKS - 1)),
        )

    def emit_uc(Wsb, Wps, Usb, first, last):
        nc.vector.tensor_copy(out=Wsb[:], in_=Wps[:])
        nc.tensor.matmul(
            Up0[:], lhsT=Wsb[:], rhs=Pb[:, 0:512], start=True, stop=True
        )
        nc.tensor.matmul(
            Up1[:], lhsT=Wsb[:], rhs=Pb[:, 512:1024], start=True, stop=True
        )
        if not last:
            # phase still running: keep DVE free, ACT absorbs both copies
            nc.scalar.copy(out=Usb[:, 0:512], in_=Up0[:])
            nc.scalar.copy(out=Usb[:, 512:1024], in_=Up1[:])
        else:
            nc.vector.tensor_copy(out=Usb[:, 0:512], in_=Up0[:])
            nc.scalar.copy(out=Usb[:, 512:1024], in_=Up1[:])
        for b in range(B):
            sl = slice(b * 128, (b + 1) * 128)
            nc.tensor.matmul(
                Cps[:],
                lhsT=Usb[:, sl],
                rhs=Pb[:, sl],
                start=(first and b == 0),
                stop=(last and b == B - 1),
            )

    pb_at = (18, 20) if SPLIT_W else (40, 43)
    for c in range(CHUNKS):
        if c == pb_at[0]:
            # P f32->bf16 slipped into the one-hot stream, in time for the
            # first U matmuls
            nc.vector.tensor_copy(out=Pb[:, 0:512], in_=Pf[:, 0:512])
        if c == pb_at[1]:
            nc.gpsimd.tensor_copy(out=Pb[:, 512:1024], in_=Pf[:, 512:1024])
        emit_chunk(c)
        if SPLIT_W and c == HALF - 1:
            emit_uc(WsbA, WpsA, UsbA, first=True, last=False)
    if SPLIT_W:
        emit_uc(WsbB, WpsB, UsbB, first=False, last=True)
    else:
        emit_uc(WsbA, WpsA, UsbA, first=True, last=True)

    # A_scaled = -(1/8) * (d_hw == 1); folds sign + batch-mean
    nc.gpsimd.tensor_scalar(
        out=Asc[:], in0=dsb, scalar1=1, scalar2=-0.125, op0=EQ, op1=MUL
    )

    # ---- partials: [ <C, -A/8> , sum(w) ] ----
    nc.vector.tensor_tensor(out=scr[:], in0=Cps[:], in1=Asc[:], op=MUL)
    nc.vector.tensor_reduce(
        out=prt[:, 0:1], in_=scr[:], axis=mybir.AxisListType.X, op=ADD
    )
    # partition + cross-core reduction of the [128,2] partials on host
    nc.sync.dma_start(out=o_d.ap(), in_=prt[:])


def _build(reps=1):
    import concourse.bacc as bacc
    import concourse.mybir as mybir
    import concourse.tile as tile

    f32 = mybir.dt.float32
    i16 = mybir.dt.int16

    nc = bacc.Bacc("TRN2", target_bir_lowering=False, debug=False, num_devices=NCORES)

    P_d = nc.dram_tensor("p_in", [B, NL, NQ], f32, kind="ExternalInput")
    meta_d = nc.dram_tensor("meta_in", [128, META_W], i16, kind="ExternalInput")
    o_d = nc.dram_tensor("out", [128, 2], f32, kind="ExternalOutput")

    with tile.TileContext(nc) as tc:
        with (
            tc.tile_pool(name="sbuf", bufs=1) as sp,
            tc.tile_pool(name="psum", bufs=1, space="PSUM") as pp,
        ):
            for _ in range(reps):
                _emit_body(nc, sp, pp, (P_d, meta_d, o_d))

    nc.compile()
    return nc


def _get_built():
    global _BUILT
    if _BUILT is None:
        _BUILT = _build()
    return _BUILT


def _shard_inputs(P, d_hw, circuit_edge_pairs, circuit_edge_weights):
    P = np.ascontiguousarray(np.asarray(P, dtype=np.float32))
    d_hw = np.ascontiguousarray(np.asarray(d_hw, dtype=np.int32))
    pairs = np.asarray(circuit_edge_pairs).astype(np.int64, copy=False)
    w = np.asarray(circuit_edge_weights, dtype=np.float32)

    pairs_pad = np.zeros((NCORES, EPAD, 2), dtype=np.int64)
    w_pad = np.zeros((NCORES, EPAD), dtype=np.float32)
    pairs_pad[:, :ESH] = pairs.reshape(NCORES, ESH, 2)
    w_pad[:, :ESH] = w.reshape(NCORES, ESH)

    # packed per-partition row: 392 i16 of pairs | 98 i16 (49 f32 w) |
    # 256 i16 (128 i32 d row) | pad to 768
    meta = np.zeros((NCORES, 128, META_W), dtype=np.int16)
    meta[:, :, 0:392] = pairs_pad.view(np.int16).reshape(NCORES, 128, 392)
    meta[:, :, 392:490] = w_pad.view(np.int16).reshape(NCORES, 128, 98)
    meta[:, :, 490:746] = d_hw.view(np.int16).reshape(128, 256)[None]

    return [
        {"p_in": P, "meta_in": np.ascontiguousarray(meta[i])}
        for i in range(NCORES)
    ]


def _combine(results):
    parts = np.stack([np.asarray(results[i]["out"]) for i in range(NCORES)])
    numer = float(parts[:, :, 0].astype(np.float64).sum())
    wsum = float(parts[:, :, 1].astype(np.float64).sum())
    return np.float32(numer / max(wsum, 1e-8))


def kernel(P, d_hw, circuit_edge_pairs, circuit_edge_weights, _want_results=False):
    from concourse.bass_utils import run_bass_kernel_spmd

    nc = _get_built()
    in_maps = _shard_inputs(P, d_hw, circuit_edge_pairs, circuit_edge_weights)
    res = run_bass_kernel_spmd(nc, in_maps, core_ids=list(range(NCORES)))
    out = _combine(res.results)
    if _want_results:
        return out, res
    return out
